# revision 23
# baseline (speedup 1.0000x reference)
"""Trainium2 Bass kernel for nn_AE_29171417875247 (k-sparse autoencoder with
top-k masking).

  h1 = sigmoid(x @ enc_W0 + enc_b0)        [B, 2048]
  h2 = sigmoid(h1 @ enc_W1 + enc_b1)       [B, 1024]
  h2 = keep top-51 per row, zero rest      (k = 1024 * 0.05)
  d  = sigmoid(h2 @ dec_W1 + dec_b1)       [B, 2048]
  out = d @ dec_W0 + dec_b0                [B, 4096]

Data-parallel across 8 NeuronCores: each core owns 1024 rows of the batch
and the full (replicated) weights. Matmuls run in float32r (fp32 with
11-bit mantissa, full PE rate); inputs are pre-rounded to f32r on the host
(round-to-nearest-even, keep 11 mantissa bits) which matches the hardware
cast bit-exactly. Top-k selection runs on exact f32 sigmoid outputs via the
DVE max8 + match_replace instructions (7 rounds of 8, last round keeps 3),
identical semantics to jax.lax.top_k for distinct values.

Per-core dataflow (rows on the moving/free side so N=512 per matmul):
  MM1: h1T[m,rows] += W0[k,m].T @ xT[k,rows]  (W0 slabs streamed, xT resident)
       -> sigmoid+b0 -> h1T staged to DRAM (f32r)
  MM2: h2[rows,n]  += h1T[kk,rows].T @ W1[kk,n] (+ rank-1 bias matmul)
       -> sigmoid -> h2 [rows on partitions]
  topk: 7x(max8 + match_replace) -> zapped; hmask = h2 - zapped
  PE-transpose hmask -> hmaskT (f32r)
  MM3: dT[m,rows] += dW1[kk,m].T @ hmaskT[kk,rows] -> sigmoid+db1 -> dT (f32r)
  MM4: outT[m,rows] += dW0[kk,m].T @ dT[kk,rows] -> +db0 -> DRAM
Host transposes outT shards back and concatenates.
"""
import sys
sys.path.insert(0, '/opt/trn_rl_repo')
import numpy as np

B, D, H1, H2 = 8192, 4096, 2048, 1024
NCORES = 8
BC = B // NCORES          # rows per core = 1024
K_TOP = 51                # int(H2 * 0.05)
KD = D // 128             # 32 k-chunks for MM1
KH1 = H1 // 128           # 16
KH2 = H2 // 128           # 8
M1 = H1 // 128            # 16 h1 tiles
M3 = H1 // 128            # 16 dT tiles
M4 = D // 128             # 32 out tiles
NR = BC // 512            # 2 row halves of 512


def _round_f32r(x: np.ndarray) -> np.ndarray:
    """Round f32 -> f32r (keep 11 mantissa bits, round-to-nearest-even).
    Bit-exact match to the hardware's f32->f32r cast (verified on silicon)."""
    b = np.ascontiguousarray(x, dtype=np.float32).view(np.uint32).astype(np.uint64)
    shift = 23 - 11
    add = (1 << (shift - 1)) - 1 + ((b >> shift) & 1)
    out = ((b + add) >> shift) << shift
    return out.astype(np.uint32).view(np.float32)


_LDW_PATCHED = False


def _patch_ldw_opt():
    """Enable walrus --enable-ldw-opt (better LDWEIGHTS scheduling, ~6% PE
    win; output verified bit-identical on this kernel's matmul mix)."""
    global _LDW_PATCHED
    if _LDW_PATCHED:
        return
    from concourse import bass_utils
    orig = bass_utils.run_command

    def patched(cmd, **kw):
        cmd = [c.replace("--enable-ldw-opt=false", "--enable-ldw-opt=true")
               if isinstance(c, str) else c for c in cmd]
        return orig(cmd, **kw)

    bass_utils.run_command = patched
    _LDW_PATCHED = True


def _build(loop_k: int = 1, stages: str = 'ABCD'):
    import contextlib
    import concourse.bacc as bacc
    import concourse.mybir as mybir
    import concourse.tile as tile

    _patch_ldw_opt()

    f32 = mybir.dt.float32
    f32r = mybir.dt.float32r
    SIG = mybir.ActivationFunctionType.Sigmoid

    nc = bacc.Bacc("TRN2", target_bir_lowering=False, debug=False)
    XTR = nc.dram_tensor("XTR", (128, KD * BC), f32r, kind="ExternalInput").ap()
    W0R = nc.dram_tensor("W0R", (M1, 128, KD * 128), f32r, kind="ExternalInput").ap()
    W1R = nc.dram_tensor("W1R", (2, 128, KH1 * 512), f32r, kind="ExternalInput").ap()
    DW1R = nc.dram_tensor("DW1R", (M3, 128, KH2 * 128), f32r, kind="ExternalInput").ap()
    DW0R = nc.dram_tensor("DW0R", (M4, 128, KH1 * 128), f32r, kind="ExternalInput").ap()
    B1R = nc.dram_tensor("B1R", (1, H2), f32r, kind="ExternalInput").ap()
    B0R = nc.dram_tensor("B0R", (128, M1), f32, kind="ExternalInput").ap()
    DB1R = nc.dram_tensor("DB1R", (128, M3), f32, kind="ExternalInput").ap()
    DB0R = nc.dram_tensor("DB0R", (128, M4), f32, kind="ExternalInput").ap()
    IDENT = nc.dram_tensor("IDENT", (128, 128), f32, kind="ExternalInput").ap()
    ONESR = nc.dram_tensor("ONESR", (1, 128), f32r, kind="ExternalInput").ap()
    OUTT = nc.dram_tensor("OUTT", (M4, 128, BC), f32, kind="ExternalOutput").ap()
    H1T = nc.dram_tensor("H1T", (M1, 128, BC), f32r, kind="Internal").ap()

    with tile.TileContext(nc) as tc:
        loop_cm = tc.For_i(0, loop_k, 1) if loop_k > 1 else contextlib.nullcontext()
        with loop_cm, \
             tc.tile_pool(name="const", bufs=1) as constp, \
             tc.tile_pool(name="psum", bufs=6, space="PSUM") as psp:
            # merged per-partition biases: [b0 | db1 | db0]
            biases = constp.tile([128, M1 + M3 + M4], f32)
            nc.sync.dma_start(biases[:, 0:M1], B0R)
            nc.sync.dma_start(biases[:, M1:M1 + M3], DB1R)
            nc.sync.dma_start(biases[:, M1 + M3:], DB0R)
            b0t = biases[:, 0:M1]
            db1t = biases[:, M1:M1 + M3]
            db0t = biases[:, M1 + M3:M1 + M3 + M4]
            ident = constp.tile([128, 128], f32)
            nc.sync.dma_start(ident[:], IDENT)
            b1t = constp.tile([1, H2], f32r)
            nc.sync.dma_start(b1t[:], B1R)
            ones1 = constp.tile([1, 128], f32r)
            nc.sync.dma_start(ones1[:], ONESR)

            # ---- Stage A: MM1  h1T = sigmoid(W0.T @ xT + b0) -> DRAM ----
            with tc.tile_pool(name="xt", bufs=1) as xtp, \
                 tc.tile_pool(name="w0", bufs=3) as w0p, \
                 tc.tile_pool(name="h1o", bufs=2) as h1op:
                xt = xtp.tile([128, KD * BC], f32r)
                # split the 16MB load so MM1's accumulation can start as soon
                # as the first k-chunks land
                XT_SPLIT = 8
                step = KD * BC // XT_SPLIT
                for i in range(XT_SPLIT):
                    nc.scalar.dma_start(xt[:, i * step:(i + 1) * step],
                                        XTR[:, i * step:(i + 1) * step])
                for m in range(M1):
                    w0s = w0p.tile([128, KD * 128], f32r)
                    nc.sync.dma_start(w0s[:], W0R[m])
                    h1o = h1op.tile([128, BC], f32r)
                    for n in range(NR):
                        ps = psp.tile([128, 512], f32)
                        for k in range(KD):
                            nc.tensor.matmul(
                                ps[:], w0s[:, k * 128:(k + 1) * 128],
                                xt[:, k * BC + n * 512: k * BC + n * 512 + 512],
                                start=(k == 0), stop=(k == KD - 1))
                        nc.scalar.activation(h1o[:, n * 512:(n + 1) * 512], ps[:],
                                             SIG, bias=b0t[:, m:m + 1])
                    nc.scalar.dma_start(H1T[m], h1o[:])

            # ---- Stages B-D ----
            with tc.tile_pool(name="hmT", bufs=1) as hmTp, \
                 tc.tile_pool(name="dw1", bufs=3) as dw1p:
                hmT = [hmTp.tile([128, BC], f32r, tag=f"hmT{k}", name=f"hmT{k}")
                       for k in range(KH2)]

                with tc.tile_pool(name="w1", bufs=1) as w1p, \
                     tc.tile_pool(name="h1i", bufs=2) as h1ip, \
                     tc.tile_pool(name="h2", bufs=1) as h2p, \
                     tc.tile_pool(name="tk", bufs=2) as tkp, \
                     tc.tile_pool(name="mx8", bufs=2) as mxp, \
                     tc.tile_pool(name="tps", bufs=2, space="PSUM") as tpsp:
                    h2tiles = [h2p.tile([128, H2], f32, tag=f"h2_{r}",
                                        name=f"h2_{r}")
                               for r in range(NR * 4)]
                    w1h = w1p.tile([128, 2 * KH1 * 512], f32r)
                    if 'B' in stages:
                        for n in range(2):
                            nc.sync.dma_start(
                                w1h[:, n * KH1 * 512:(n + 1) * KH1 * 512], W1R[n])
                    for r in range(NR * 4 if 'B' in stages else 0):
                        h1s = h1ip.tile([128, KH1 * 128], f32r, tag="h1s")
                        nc.sync.dma_start(
                            h1s[:].rearrange("p (m f) -> p m f", m=KH1),
                            H1T[:, :, r * 128:(r + 1) * 128]
                            .rearrange("m p f -> p m f"))
                        for n in range(2):
                            ps = psp.tile([128, 512], f32)
                            for kk in range(KH1):
                                nc.tensor.matmul(
                                    ps[:], h1s[:, kk * 128:(kk + 1) * 128],
                                    w1h[:, (n * KH1 + kk) * 512:
                                        (n * KH1 + kk) * 512 + 512],
                                    start=(kk == 0), stop=False)
                            nc.tensor.matmul(ps[:], ones1[:],
                                             b1t[:, n * 512:(n + 1) * 512],
                                             start=False, stop=True)
                            nc.scalar.activation(
                                h2tiles[r][:, n * 512:(n + 1) * 512], ps[:], SIG)
                        # top-51 mask for row tile r
                        h2r = h2tiles[r]
                        zap = tkp.tile([128, H2], f32, tag="zap")
                        cur = h2r
                        for it in range(7):
                            mx = mxp.tile([128, 8], f32, tag="mx")
                            nc.vector.max(mx[:], cur[:])
                            if it == 6:
                                nc.vector.memset(mx[:, 3:8], 0.0)
                            nc.vector.match_replace(
                                out=zap[:], in_to_replace=mx[:],
                                in_values=cur[:], imm_value=0.0)
                            cur = zap
                        hmask = tkp.tile([128, H2], f32, tag="hmask")
                        nc.vector.tensor_sub(hmask[:], h2r[:], zap[:])
                        for kk in range(KH2):
                            pst = tpsp.tile([128, 128], f32)
                            nc.tensor.transpose(
                                pst[:], hmask[:, kk * 128:(kk + 1) * 128],
                                ident[:])
                            nc.scalar.copy(
                                hmT[kk][:, r * 128:(r + 1) * 128], pst[:])

                # ---- Stage C: MM3  dT = sigmoid(dW1.T @ hmaskT + db1) ----
                with tc.tile_pool(name="dT", bufs=1) as dTp:
                    dT = [dTp.tile([128, BC], f32r, tag=f"dT{m}", name=f"dT{m}")
                          for m in range(M3)]
                    for m in range(M3 if 'C' in stages else 0):
                        dw1s = dw1p.tile([128, KH2 * 128], f32r)
                        nc.sync.dma_start(dw1s[:], DW1R[m])
                        for n2 in range(NR):
                            ps = psp.tile([128, 512], f32)
                            for kk in range(KH2):
                                nc.tensor.matmul(
                                    ps[:], dw1s[:, kk * 128:(kk + 1) * 128],
                                    hmT[kk][:, n2 * 512:(n2 + 1) * 512],
                                    start=(kk == 0), stop=(kk == KH2 - 1))
                            nc.scalar.activation(
                                dT[m][:, n2 * 512:(n2 + 1) * 512], ps[:],
                                SIG, bias=db1t[:, m:m + 1])

                    # ---- Stage D: MM4  outT = dW0.T @ dT + db0 -> DRAM ----
                    with tc.tile_pool(name="dw0", bufs=2) as dw0p, \
                         tc.tile_pool(name="outp", bufs=3) as outp:
                        for m in range(M4 if 'D' in stages else 0):
                            dw0s = dw0p.tile([128, KH1 * 128], f32r)
                            nc.sync.dma_start(dw0s[:], DW0R[m])
                            om = outp.tile([128, BC], f32)
                            for n2 in range(NR):
                                ps = psp.tile([128, 512], f32)
                                for kk in range(KH1):
                                    nc.tensor.matmul(
                                        ps[:], dw0s[:, kk * 128:(kk + 1) * 128],
                                        dT[kk][:, n2 * 512:(n2 + 1) * 512],
                                        start=(kk == 0), stop=(kk == KH1 - 1))
                                nc.vector.tensor_scalar_add(
                                    om[:, n2 * 512:(n2 + 1) * 512], ps[:],
                                    db0t[:, m:m + 1])
                            nc.scalar.dma_start(OUTT[m], om[:])
    nc.compile()
    return nc




_NC_CACHE = None


def _get_nc():
    global _NC_CACHE
    if _NC_CACHE is None:
        _NC_CACHE = _build()
    return _NC_CACHE


def _build_looped(loop_k: int):
    return _build(loop_k)


def make_in_maps(x, enc_W0, enc_b0, enc_W1, enc_b1, dec_W1, dec_b1, dec_W0,
                 dec_b0):
    w0r = _round_f32r(enc_W0).reshape(KD, 128, M1, 128) \
        .transpose(2, 1, 0, 3).reshape(M1, 128, KD * 128)
    w1r = _round_f32r(enc_W1).reshape(KH1, 128, 2, 512) \
        .transpose(2, 1, 0, 3).reshape(2, 128, KH1 * 512)
    dw1r = _round_f32r(dec_W1).reshape(KH2, 128, M3, 128) \
        .transpose(2, 1, 0, 3).reshape(M3, 128, KH2 * 128)
    dw0r = _round_f32r(dec_W0).reshape(KH1, 128, M4, 128) \
        .transpose(2, 1, 0, 3).reshape(M4, 128, KH1 * 128)
    b1r = _round_f32r(enc_b1).reshape(1, H2)
    b0r = np.ascontiguousarray(enc_b0.reshape(M1, 128).T)
    db1r = np.ascontiguousarray(dec_b1.reshape(M3, 128).T)
    db0r = np.ascontiguousarray(dec_b0.reshape(M4, 128).T)
    ident = np.eye(128, dtype=np.float32)
    shared = dict(W0R=np.ascontiguousarray(w0r), W1R=np.ascontiguousarray(w1r),
                  DW1R=np.ascontiguousarray(dw1r),
                  DW0R=np.ascontiguousarray(dw0r), B1R=b1r, B0R=b0r,
                  DB1R=db1r, DB0R=db0r, IDENT=ident,
                  ONESR=np.ones((1, 128), dtype=np.float32))
    xr = _round_f32r(x)
    in_maps = []
    for c in range(NCORES):
        shard = xr[c * BC:(c + 1) * BC]          # [BC, D]
        xt = np.ascontiguousarray(
            shard.T.reshape(KD, 128, BC).transpose(1, 0, 2)
        ).reshape(128, KD * BC)
        in_maps.append(dict(shared, XTR=xt))
    return in_maps


def kernel(**inputs) -> np.ndarray:
    from concourse import bass_utils
    nc = _get_nc()
    in_maps = make_in_maps(**inputs)
    res = bass_utils.run_bass_kernel_spmd(nc, in_maps,
                                          core_ids=list(range(NCORES)))
    outs = []
    for c in range(NCORES):
        ot = res.results[c]["OUTT"]              # [M4, 128, BC]
        outs.append(ot.reshape(D, BC).T)         # [BC, D]
    return np.ascontiguousarray(np.concatenate(outs, axis=0), dtype=np.float32)


# revision 24
# speedup vs baseline: 1.1610x; 1.1610x over previous
"""Trainium2 Bass kernel for nn_AE_29171417875247 (k-sparse autoencoder with
top-k masking).

  h1 = sigmoid(x @ enc_W0 + enc_b0)        [B, 2048]
  h2 = sigmoid(h1 @ enc_W1 + enc_b1)       [B, 1024]
  h2 = keep top-51 per row, zero rest      (k = 1024 * 0.05)
  d  = sigmoid(h2 @ dec_W1 + dec_b1)       [B, 2048]
  out = d @ dec_W0 + dec_b0                [B, 4096]

Data-parallel across 8 NeuronCores: each core owns 1024 rows of the batch
and the full (replicated) weights. Matmuls run in float32r (fp32 with
11-bit mantissa, full PE rate); inputs are pre-rounded to f32r on the host
(round-to-nearest-even, keep 11 mantissa bits) which matches the hardware
cast bit-exactly. Top-k selection runs on exact f32 sigmoid outputs via the
DVE max8 + match_replace instructions (7 rounds of 8, last round keeps 3),
identical semantics to jax.lax.top_k for distinct values.

Per-core dataflow (rows on the moving/free side so N=512 per matmul):
  MM1: h1T[m,rows] += W0[k,m].T @ xT[k,rows]  (W0 slabs streamed, xT resident)
       -> sigmoid+b0 -> h1T staged to DRAM (f32r)
  MM2: h2[rows,n]  += h1T[kk,rows].T @ W1[kk,n] (+ rank-1 bias matmul)
       -> sigmoid -> h2 [rows on partitions]
  topk: 7x(max8 + match_replace) -> zapped; hmask = h2 - zapped
  PE-transpose hmask -> hmaskT (f32r)
  MM3: dT[m,rows] += dW1[kk,m].T @ hmaskT[kk,rows] -> sigmoid+db1 -> dT (f32r)
  MM4: outT[m,rows] += dW0[kk,m].T @ dT[kk,rows] -> +db0 -> DRAM
Host transposes outT shards back and concatenates.
"""
import sys
sys.path.insert(0, '/opt/trn_rl_repo')
import numpy as np

B, D, H1, H2 = 8192, 4096, 2048, 1024
NCORES = 8
BC = B // NCORES          # rows per core = 1024
K_TOP = 51                # int(H2 * 0.05)
KD = D // 128             # 32 k-chunks for MM1
KH1 = H1 // 128           # 16
KH2 = H2 // 128           # 8
M1 = H1 // 128            # 16 h1 tiles
M3 = H1 // 128            # 16 dT tiles
M4 = D // 128             # 32 out tiles
NR = BC // 512            # 2 row halves of 512


def _round_f32r(x: np.ndarray) -> np.ndarray:
    """Round f32 -> f32r (keep 11 mantissa bits, round-to-nearest-even).
    Bit-exact match to the hardware's f32->f32r cast (verified on silicon)."""
    b = np.ascontiguousarray(x, dtype=np.float32).view(np.uint32).astype(np.uint64)
    shift = 23 - 11
    add = (1 << (shift - 1)) - 1 + ((b >> shift) & 1)
    out = ((b + add) >> shift) << shift
    return out.astype(np.uint32).view(np.float32)


_LDW_PATCHED = False


def _patch_ldw_opt():
    """Enable walrus --enable-ldw-opt (better LDWEIGHTS scheduling, ~6% PE
    win; output verified bit-identical on this kernel's matmul mix)."""
    global _LDW_PATCHED
    if _LDW_PATCHED:
        return
    from concourse import bass_utils
    orig = bass_utils.run_command

    def patched(cmd, **kw):
        cmd = [c.replace("--enable-ldw-opt=false", "--enable-ldw-opt=true")
               if isinstance(c, str) else c for c in cmd]
        return orig(cmd, **kw)

    bass_utils.run_command = patched
    _LDW_PATCHED = True


def _build(loop_k: int = 1, stages: str = 'ABCD'):
    import contextlib
    import concourse.bacc as bacc
    import concourse.mybir as mybir
    import concourse.tile as tile

    _patch_ldw_opt()

    f32 = mybir.dt.float32
    f32r = mybir.dt.float32r
    SIG = mybir.ActivationFunctionType.Sigmoid

    nc = bacc.Bacc("TRN2", target_bir_lowering=False, debug=False)
    XTR = nc.dram_tensor("XTR", (128, KD * BC), f32r, kind="ExternalInput").ap()
    W0R = nc.dram_tensor("W0R", (M1, 128, KD * 128), f32r, kind="ExternalInput").ap()
    W1R = nc.dram_tensor("W1R", (2, 128, KH1 * 512), f32r, kind="ExternalInput").ap()
    DW1R = nc.dram_tensor("DW1R", (M3, 128, KH2 * 128), f32r, kind="ExternalInput").ap()
    DW0R = nc.dram_tensor("DW0R", (M4, 128, KH1 * 128), f32r, kind="ExternalInput").ap()
    B1R = nc.dram_tensor("B1R", (1, H2), f32r, kind="ExternalInput").ap()
    B0R = nc.dram_tensor("B0R", (128, M1), f32, kind="ExternalInput").ap()
    DB1R = nc.dram_tensor("DB1R", (128, M3), f32, kind="ExternalInput").ap()
    DB0R = nc.dram_tensor("DB0R", (128, M4), f32, kind="ExternalInput").ap()
    IDENT = nc.dram_tensor("IDENT", (128, 128), f32, kind="ExternalInput").ap()
    ONESR = nc.dram_tensor("ONESR", (1, 128), f32r, kind="ExternalInput").ap()
    OUTT = nc.dram_tensor("OUTT", (M4, 128, BC), f32, kind="ExternalOutput").ap()
    H1T = nc.dram_tensor("H1T", (M1, 128, BC), f32r, kind="Internal").ap()

    with tile.TileContext(nc) as tc:
        loop_cm = tc.For_i(0, loop_k, 1) if loop_k > 1 else contextlib.nullcontext()
        with loop_cm, \
             tc.tile_pool(name="const", bufs=1) as constp, \
             tc.tile_pool(name="psum", bufs=6, space="PSUM") as psp:
            # merged per-partition biases: [b0 | db1 | db0]
            biases = constp.tile([128, M1 + M3 + M4], f32)
            nc.sync.dma_start(biases[:, 0:M1], B0R)
            nc.sync.dma_start(biases[:, M1:M1 + M3], DB1R)
            nc.sync.dma_start(biases[:, M1 + M3:], DB0R)
            b0t = biases[:, 0:M1]
            db1t = biases[:, M1:M1 + M3]
            db0t = biases[:, M1 + M3:M1 + M3 + M4]
            ident = constp.tile([128, 128], f32)
            nc.sync.dma_start(ident[:], IDENT)
            b1t = constp.tile([1, H2], f32r)
            nc.sync.dma_start(b1t[:], B1R)
            ones1 = constp.tile([1, 128], f32r)
            nc.sync.dma_start(ones1[:], ONESR)

            # ---- Stage A: MM1  h1T = sigmoid(W0.T @ xT + b0) -> DRAM ----
            with tc.tile_pool(name="xt", bufs=1) as xtp, \
                 tc.tile_pool(name="w0", bufs=2) as w0p, \
                 tc.tile_pool(name="h1o", bufs=2) as h1op:
                xt = xtp.tile([128, KD * BC], f32r)
                # split the 16MB load so MM1's accumulation can start as soon
                # as the first k-chunks land
                XT_SPLIT = 8
                step = KD * BC // XT_SPLIT
                for i in range(XT_SPLIT):
                    nc.scalar.dma_start(xt[:, i * step:(i + 1) * step],
                                        XTR[:, i * step:(i + 1) * step])
                for m in range(M1):
                    w0s = w0p.tile([128, KD * 128], f32r)
                    nc.sync.dma_start(w0s[:], W0R[m])
                    h1o = h1op.tile([128, BC], f32r)
                    for n in range(NR):
                        ps = psp.tile([128, 512], f32)
                        for k in range(KD):
                            nc.tensor.matmul(
                                ps[:], w0s[:, k * 128:(k + 1) * 128],
                                xt[:, k * BC + n * 512: k * BC + n * 512 + 512],
                                start=(k == 0), stop=(k == KD - 1))
                        nc.scalar.activation(h1o[:, n * 512:(n + 1) * 512], ps[:],
                                             SIG, bias=b0t[:, m:m + 1])
                    nc.scalar.dma_start(H1T[m], h1o[:])

            # ---- Stages B-D ----
            with tc.tile_pool(name="hmT", bufs=1) as hmTp, \
                 tc.tile_pool(name="dw1", bufs=3) as dw1p:
                hmT = [hmTp.tile([128, BC], f32r, tag=f"hmT{k}", name=f"hmT{k}")
                       for k in range(KH2)]

                with tc.tile_pool(name="w1", bufs=1) as w1p, \
                     tc.tile_pool(name="h1i", bufs=2) as h1ip, \
                     tc.tile_pool(name="h2", bufs=1) as h2p, \
                     tc.tile_pool(name="tk", bufs=2) as tkp, \
                     tc.tile_pool(name="mx8", bufs=2) as mxp, \
                     tc.tile_pool(name="tps", bufs=2, space="PSUM") as tpsp:
                    h2tiles = [h2p.tile([128, H2], f32, tag=f"h2_{r}",
                                        name=f"h2_{r}")
                               for r in range(NR * 4)]
                    w1h = w1p.tile([128, 2 * KH1 * 512], f32r)
                    if 'B' in stages:
                        for n in range(2):
                            nc.sync.dma_start(
                                w1h[:, n * KH1 * 512:(n + 1) * KH1 * 512], W1R[n])
                    for r in range(NR * 4 if 'B' in stages else 0):
                        h1s = h1ip.tile([128, KH1 * 128], f32r, tag="h1s")
                        nc.sync.dma_start(
                            h1s[:].rearrange("p (m f) -> p m f", m=KH1),
                            H1T[:, :, r * 128:(r + 1) * 128]
                            .rearrange("m p f -> p m f"))
                        for n in range(2):
                            ps = psp.tile([128, 512], f32)
                            for kk in range(KH1):
                                nc.tensor.matmul(
                                    ps[:], h1s[:, kk * 128:(kk + 1) * 128],
                                    w1h[:, (n * KH1 + kk) * 512:
                                        (n * KH1 + kk) * 512 + 512],
                                    start=(kk == 0), stop=False)
                            nc.tensor.matmul(ps[:], ones1[:],
                                             b1t[:, n * 512:(n + 1) * 512],
                                             start=False, stop=True)
                            nc.scalar.activation(
                                h2tiles[r][:, n * 512:(n + 1) * 512], ps[:], SIG)
                        # top-51 mask for row tile r
                        h2r = h2tiles[r]
                        zap = tkp.tile([128, H2], f32, tag="zap")
                        cur = h2r
                        for it in range(7):
                            mx = mxp.tile([128, 8], f32, tag="mx")
                            nc.vector.max(mx[:], cur[:])
                            if it == 6:
                                nc.vector.memset(mx[:, 3:8], 0.0)
                            nc.vector.match_replace(
                                out=zap[:], in_to_replace=mx[:],
                                in_values=cur[:], imm_value=0.0)
                            cur = zap
                        hmask = tkp.tile([128, H2], f32, tag="hmask")
                        nc.vector.tensor_sub(hmask[:], h2r[:], zap[:])
                        for kk in range(KH2):
                            pst = tpsp.tile([128, 128], f32)
                            nc.tensor.transpose(
                                pst[:], hmask[:, kk * 128:(kk + 1) * 128],
                                ident[:])
                            nc.scalar.copy(
                                hmT[kk][:, r * 128:(r + 1) * 128], pst[:])

                # ---- Stage C: MM3  dT = sigmoid(dW1.T @ hmaskT + db1) ----
                with tc.tile_pool(name="dT", bufs=1) as dTp:
                    dT = [dTp.tile([128, BC], f32r, tag=f"dT{m}", name=f"dT{m}")
                          for m in range(M3)]
                    for m in range(M3 if 'C' in stages else 0):
                        dw1s = dw1p.tile([128, KH2 * 128], f32r)
                        nc.sync.dma_start(dw1s[:], DW1R[m])
                        for n2 in range(NR):
                            ps = psp.tile([128, 512], f32)
                            for kk in range(KH2):
                                nc.tensor.matmul(
                                    ps[:], dw1s[:, kk * 128:(kk + 1) * 128],
                                    hmT[kk][:, n2 * 512:(n2 + 1) * 512],
                                    start=(kk == 0), stop=(kk == KH2 - 1))
                            nc.scalar.activation(
                                dT[m][:, n2 * 512:(n2 + 1) * 512], ps[:],
                                SIG, bias=db1t[:, m:m + 1])

                    # ---- Stage D: MM4  outT = dW0.T @ dT + db0 -> DRAM ----
                    with tc.tile_pool(name="dw0", bufs=2) as dw0p, \
                         tc.tile_pool(name="outp", bufs=3) as outp:
                        for m in range(M4 if 'D' in stages else 0):
                            dw0s = dw0p.tile([128, KH1 * 128], f32r)
                            nc.sync.dma_start(dw0s[:], DW0R[m])
                            om = outp.tile([128, BC], f32)
                            for n2 in range(NR):
                                ps = psp.tile([128, 512], f32)
                                for kk in range(KH1):
                                    nc.tensor.matmul(
                                        ps[:], dw0s[:, kk * 128:(kk + 1) * 128],
                                        dT[kk][:, n2 * 512:(n2 + 1) * 512],
                                        start=(kk == 0), stop=(kk == KH1 - 1))
                                nc.vector.tensor_scalar_add(
                                    om[:, n2 * 512:(n2 + 1) * 512], ps[:],
                                    db0t[:, m:m + 1])
                            nc.scalar.dma_start(OUTT[m], om[:])
    nc.compile()
    return nc




_NC_CACHE = None


def _get_nc():
    global _NC_CACHE
    if _NC_CACHE is None:
        _NC_CACHE = _build()
    return _NC_CACHE


def _build_looped(loop_k: int):
    return _build(loop_k)


def make_in_maps(x, enc_W0, enc_b0, enc_W1, enc_b1, dec_W1, dec_b1, dec_W0,
                 dec_b0):
    w0r = _round_f32r(enc_W0).reshape(KD, 128, M1, 128) \
        .transpose(2, 1, 0, 3).reshape(M1, 128, KD * 128)
    w1r = _round_f32r(enc_W1).reshape(KH1, 128, 2, 512) \
        .transpose(2, 1, 0, 3).reshape(2, 128, KH1 * 512)
    dw1r = _round_f32r(dec_W1).reshape(KH2, 128, M3, 128) \
        .transpose(2, 1, 0, 3).reshape(M3, 128, KH2 * 128)
    dw0r = _round_f32r(dec_W0).reshape(KH1, 128, M4, 128) \
        .transpose(2, 1, 0, 3).reshape(M4, 128, KH1 * 128)
    b1r = _round_f32r(enc_b1).reshape(1, H2)
    b0r = np.ascontiguousarray(enc_b0.reshape(M1, 128).T)
    db1r = np.ascontiguousarray(dec_b1.reshape(M3, 128).T)
    db0r = np.ascontiguousarray(dec_b0.reshape(M4, 128).T)
    ident = np.eye(128, dtype=np.float32)
    shared = dict(W0R=np.ascontiguousarray(w0r), W1R=np.ascontiguousarray(w1r),
                  DW1R=np.ascontiguousarray(dw1r),
                  DW0R=np.ascontiguousarray(dw0r), B1R=b1r, B0R=b0r,
                  DB1R=db1r, DB0R=db0r, IDENT=ident,
                  ONESR=np.ones((1, 128), dtype=np.float32))
    xr = _round_f32r(x)
    in_maps = []
    for c in range(NCORES):
        shard = xr[c * BC:(c + 1) * BC]          # [BC, D]
        xt = np.ascontiguousarray(
            shard.T.reshape(KD, 128, BC).transpose(1, 0, 2)
        ).reshape(128, KD * BC)
        in_maps.append(dict(shared, XTR=xt))
    return in_maps


def kernel(**inputs) -> np.ndarray:
    from concourse import bass_utils
    nc = _get_nc()
    in_maps = make_in_maps(**inputs)
    res = bass_utils.run_bass_kernel_spmd(nc, in_maps,
                                          core_ids=list(range(NCORES)))
    outs = []
    for c in range(NCORES):
        ot = res.results[c]["OUTT"]              # [M4, 128, BC]
        outs.append(ot.reshape(D, BC).T)         # [BC, D]
    return np.ascontiguousarray(np.concatenate(outs, axis=0), dtype=np.float32)
